# revision 16
# baseline (speedup 1.0000x reference)
"""DLASSO-GNN-Hypernet ADMM forward on 8 Trainium2 NeuronCores (Bass/Tile).

Sharding: data-parallel over batch (8 samples/core). Graph message passing is
done as dense per-sample [50,50] operators built host-side from the integer
edge lists; float compute runs on-device mostly in fp16 (PE matmuls run 2x
faster than fp32 and fp16's 10-bit mantissa keeps the end-to-end rel-err
~5e-3, measured against the fp32 oracle on CPU). The Laplacian/state/gradient
path stays fp32. Wd1 is streamed from HBM as fp16 (26MB/step) through a deep
ring buffer so the stream prefetches during the GCN front phase.

Layouts per core (8 samples, 2-sample "packs", pack rows r = b_loc*50 + p):
  transposed activations: [feat, col] with col = p*8 + b   (feature on SBUF
  partitions; per-node column groups of 8 contiguous for per-node matmuls)
  natural ADMM states:    4 tiles [100, 200] (pack-stacked rows)
"""
import numpy as np

import concourse.bass as bass
import concourse.bacc as bacc
import concourse.tile as tile
import concourse.mybir as mybir
from concourse import bass_utils

F32 = mybir.dt.float32
F16 = mybir.dt.float16
AF = mybir.ActivationFunctionType
ALU = mybir.AluOpType

B, P, M, N, H, K = 64, 50, 100, 200, 128, 5
BC = 8                      # samples per core
NPACK, PKW = 4, 100         # packs of 2 samples -> 100 rows each
NC_CORES = 8
FOUT = [128, 256, 512, 512, 512]        # GCN layer output dims
FIN = [400, 128, 256, 512, 512]
LN_EPS = 1e-5
RING_BUFS = 12

KT400 = [(0, 128), (128, 72), (200, 128), (328, 72)]
KT200 = [(0, 128), (128, 72)]


def _kt(dim):
    if dim == 400:
        return KT400
    return [(s, min(128, dim - s)) for s in range(0, dim, 128)]


def _pk(ap_2d, pack):
    """[*, 400]-col AP (col = b*50+p) -> pack's contiguous 100 cols."""
    return ap_2d[:, PKW * pack:PKW * pack + PKW]


def _nd(ap_2d, p):
    """[*, 400]-col AP (col = b*50+p) -> node p's 8 sample-cols (stride 50)."""
    return ap_2d.rearrange("a (b p) -> a p b", p=P)[:, p, :]


def _newton_rsqrt(nc, pool, t_ap, shape, tag):
    """rstd = 1/sqrt(t). ACT Sqrt + DVE reciprocal seed + 1 Newton step."""
    s0 = pool.tile(shape, F32, tag=tag + "s0", name=tag + "s0")
    nc.scalar.activation(s0[:], t_ap, AF.Sqrt)
    r0 = pool.tile(shape, F32, tag=tag + "r0", name=tag + "r0")
    nc.vector.reciprocal(r0[:], s0[:])
    r = pool.tile(shape, F32, tag=tag + "r", name=tag + "r")
    tmp = pool.tile(shape, F32, tag=tag + "t", name=tag + "t")
    nc.vector.tensor_mul(tmp[:], r0[:], r0[:])
    nc.vector.tensor_mul(tmp[:], tmp[:], t_ap)
    nc.vector.tensor_scalar(tmp[:], tmp[:], -0.5, 1.5, ALU.mult, ALU.add)
    nc.vector.tensor_mul(r[:], r0[:], tmp[:])
    return r


def build_nc(skip_bv=True, skip_ld=True, skip_bd=True):
    """skip_* : omit device work for bias/affine params that are all-zero /
    identity in the given inputs (decided host-side)."""
    nc = bacc.Bacc("TRN2", target_bir_lowering=False, debug=False,
                   enable_asserts=False, num_devices=NC_CORES)

    def din(name, shape, dt=F16):
        return nc.dram_tensor(name, list(shape), dt, kind="ExternalInput").ap()

    d_AtA = din("AtA32", (N, P * N), F32)
    d_Atb_n = din("Atbn", (NPACK * PKW, N), F32)
    d_AtbT = din("AtbT", (N, BC * P))
    d_y0T = din("y0T", (N, BC * P), F32)
    d_y0n = din("y0n", (NPACK * PKW, N), F32)
    d_U0n = din("U0n", (NPACK * PKW, N), F32)
    d_d0n = din("d0n", (NPACK * PKW, N), F32)
    d_Gt = din("GhatT", (NPACK * PKW, PKW))
    d_Gte = din("GhatE", (NPACK * 2, PKW))
    d_Lt = din("LdT", (NPACK * PKW, PKW), F32)
    d_sel = din("sel", (BC, NPACK * PKW), F32)
    d_snb = din("snb", (PKW, NPACK), F32)
    d_id = din("ident", (128, 128))          # f16 identity for transposes
    d_W = [din(f"W{l+1}f", (FIN[l], FOUT[l])) for l in range(5)]
    d_bv = [din(f"bv{l+1}", (2, FOUT[l])) for l in range(5)]
    d_Wd1 = din("Wd1", (P * 512, 512))
    d_Wd2 = din("Wd2", (512, 256), F32)
    d_Wd3 = din("Wd3", (256, 128), F32)
    d_Wfc = din("Wfc", (128, 4), F32)
    d_bdr = [din(n, (1, f), F32) for n, f in
             (("bd1r", 512), ("bd2r", 256), ("bd3r", 128), ("bfcr", 4))]
    d_ldg = [din(n, (BC, f), F32) for n, f in
             (("ld1gb", 512), ("ld2gb", 256), ("ld3gb", 128))]
    d_ldb = [din(n, (BC, f), F32) for n, f in
             (("ld1bb", 512), ("ld2bb", 256), ("ld3bb", 128))]
    d_g5 = din("gs5c", (128, 4), F32)
    d_b5 = din("be5c", (128, 4), F32)
    d_lng = din("lngc", (128, 4), F32)
    d_lnb = din("lnbc", (128, 4), F32)
    d_out = nc.dram_tensor("Ys", [K, NPACK * PKW, N], F32,
                           kind="ExternalOutput").ap()

    with tile.TileContext(nc) as tc:
        from contextlib import ExitStack
        es = ExitStack()
        with es:
            cp = es.enter_context(tc.tile_pool(name="consts", bufs=1))

            def load(dram_ap, shape, tag, dt=F16):
                t = cp.tile(list(shape), dt, tag=tag, name=tag)
                nc.sync.dma_start(out=t[:], in_=dram_ap)
                return t

            ident = load(d_id, (128, 128), "ident")
            Gt = [load(d_Gt[PKW * m:PKW * m + PKW, :], (PKW, PKW), f"Gt{m}")
                  for m in range(NPACK)]
            Gte = [load(d_Gte[2 * m:2 * m + 2, :], (2, PKW), f"Gte{m}")
                   for m in range(NPACK)]
            Lt = [load(d_Lt[PKW * m:PKW * m + PKW, :], (PKW, PKW), f"Lt{m}",
                       F32) for m in range(NPACK)]
            sel = load(d_sel, (BC, NPACK * PKW), "sel", F32)
            snb = load(d_snb, (PKW, NPACK), "snb", F32)
            Wt = {}
            for l in range(5):
                for ki, (ks, kz) in enumerate(_kt(FIN[l])):
                    Wt[(l, ki)] = load(d_W[l][ks:ks + kz, :], (kz, FOUT[l]),
                                       f"W{l}_{ki}")
            bv = (None if skip_bv else
                  [load(d_bv[l], (2, FOUT[l]), f"bv{l}") for l in range(5)])
            Wd2t = [load(d_Wd2[s:s + z, :], (z, 256), f"wd2_{s}", F32)
                    for s, z in _kt(512)]
            Wd3t = [load(d_Wd3[s:s + z, :], (z, 128), f"wd3_{s}", F32)
                    for s, z in _kt(256)]
            Wfct = load(d_Wfc, (128, 4), "wfc", F32)
            if skip_bd:
                bdr = [None, None, None, load(d_bdr[3], (1, 4), "bdr3", F32)]
            else:
                bdr = [load(d_bdr[i], d_bdr[i].shape, f"bdr{i}", F32)
                       for i in range(4)]
            ldg = ldb = None
            if not skip_ld:
                ldg = [load(d_ldg[i], d_ldg[i].shape, f"ldg{i}", F32)
                       for i in range(3)]
                ldb = [load(d_ldb[i], d_ldb[i].shape, f"ldb{i}", F32)
                       for i in range(3)]
            g5c = load(d_g5, (128, 4), "g5c", F32)
            b5c = load(d_b5, (128, 4), "b5c", F32)
            lngc = load(d_lng, (128, 4), "lngc", F32)
            lnbc = load(d_lnb, (128, 4), "lnbc", F32)

            ones_col = cp.tile([128, 1], F16, tag="ones_col")
            nc.vector.memset(ones_col[:], 1.0)
            ones_row = cp.tile([1, 128], F16, tag="ones_row")
            nc.vector.memset(ones_row[:], 1.0)
            ones_row32 = cp.tile([1, 128], F32, tag="ones_row32")
            nc.vector.memset(ones_row32[:], 1.0)

            yT = [cp.tile([128, BC * P], F32, tag="yT0", name="yT0"),
                  cp.tile([72, BC * P], F32, tag="yT1", name="yT1")]
            nc.sync.dma_start(out=yT[0][:], in_=d_y0T[0:128, :])
            nc.sync.dma_start(out=yT[1][:], in_=d_y0T[128:200, :])
            y_nat, U_nat, dl_nat = [
                [cp.tile([PKW, N], F32, tag=f"{nm}{m}", name=f"{nm}{m}")
                 for m in range(NPACK)]
                for nm in ("y", "U", "dl")]
            Atb_nat = [cp.tile([PKW, N], F32, tag=f"Atb{m}", name=f"Atb{m}")
                       for m in range(NPACK)]
            for m in range(NPACK):
                for tl, src in ((y_nat, d_y0n), (U_nat, d_U0n), (dl_nat, d_d0n)):
                    nc.sync.dma_start(out=tl[m][:],
                                      in_=src[PKW * m:PKW * m + PKW, :])
            xnt = [cp.tile([z, BC * P], F16, tag=f"xnt{i}", name=f"xnt{i}")
                   for i, (s, z) in enumerate(KT400)]
            xnt32 = [cp.tile([z, BC * P], F32, tag=f"x32_{i}", name=f"x32_{i}")
                     for i, (s, z) in enumerate(KT200)]
            ident32 = cp.tile([128, 128], F32, tag="ident32")
            nc.vector.tensor_copy(ident32[:], ident[:])
            hT = [cp.tile([128, BC * P], F16, tag=f"h{i}", name=f"h{i}")
                  for i in range(4)]          # also holds enc^T after LN
            AtA = [cp.tile([128, P * N], F32, tag="ata0", name="ata0"),
                   cp.tile([72, P * N], F32, tag="ata1", name="ata1")]
            nc.sync.dma_start(out=AtA[0][:], in_=d_AtA[0:128, :])
            nc.sync.dma_start(out=AtA[1][:], in_=d_AtA[128:200, :])
            for m in range(NPACK):
                nc.sync.dma_start(out=Atb_nat[m][:],
                                  in_=d_Atb_n[PKW * m:PKW * m + PKW, :])
            nc.sync.dma_start(out=xnt[2][:], in_=d_AtbT[0:128, :])
            nc.sync.dma_start(out=xnt[3][:], in_=d_AtbT[128:200, :])

            ring = es.enter_context(tc.tile_pool(name="ring", bufs=RING_BUFS))

            # ---------------- K ADMM steps --------------------------------
            for k in range(K):
                sw = tc.alloc_tile_pool(name=f"sw{k}", bufs=1)
                # -- AtAy^T into xnt32 (fp32 for grad), cast to xnt f16 --
                GRP = 16
                with tc.tile_pool(name=f"ps_aty{k}", bufs=2,
                                  space="PSUM") as pp:
                    for i, (s, z) in enumerate(KT200):
                        for gs_ in range(0, P, GRP):
                            gn = min(GRP, P - gs_)
                            ps = pp.tile([z, GRP * BC], F32, tag=f"aty{i}",
                                         name="aty")
                            for p in range(gs_, gs_ + gn):
                                for j, (js, jz) in enumerate(KT200):
                                    nc.tensor.matmul(
                                        ps[:, BC * (p - gs_):BC * (p - gs_) + BC],
                                        lhsT=AtA[j][:, N * p + s:N * p + s + z],
                                        rhs=_nd(yT[j][:], p),
                                        start=(j == 0), stop=(j == 1))
                            nc.vector.tensor_copy(
                                xnt32[i].rearrange(
                                    "a (b p) -> a p b", p=P)[:, gs_:gs_ + gn, :],
                                ps[:, :gn * BC])
                        nc.vector.tensor_copy(xnt[i][:], xnt32[i][:])
                # -- g = AtAy - Atb (natural, via PE transpose) --
                g_t = [sw.tile([PKW, N], F32, tag="g", name=f"g{m}", bufs=4)
                       for m in range(NPACK)]
                with tc.tile_pool(name=f"ps_atr{k}", bufs=2,
                                  space="PSUM") as pp:
                    for m in range(NPACK):
                        for i, (s, z) in enumerate(KT200):
                            pt = pp.tile([PKW, 128], F32, tag="atr",
                                         name="atr")
                            nc.tensor.transpose(pt[:, :z], _pk(xnt32[i][:], m),
                                                ident32[:z, :z])
                            nc.vector.tensor_sub(g_t[m][:, s:s + z],
                                                 pt[:, :z],
                                                 Atb_nat[m][:, s:s + z])

                # -- GCN: 5 layers, transposed chain --
                cur = xnt
                with tc.tile_pool(name=f"ps_gcn{k}", bufs=2,
                                  space="PSUM") as pp, \
                     tc.tile_pool(name=f"gcnw{k}", bufs=2) as gw:
                    for l in range(5):
                        fo = FOUT[l]
                        nxt = (hT if l == 4 else
                               [gw.tile([128, BC * P], F16, tag=f"xt{i}",
                                        name=f"xt{i}")
                                for i in range((fo + 127) // 128)])
                        Fsbs = []
                        for m in range(NPACK):
                            psF = pp.tile([PKW, 512], F32, tag="F", name="F")
                            kts = _kt(FIN[l])
                            for ki, (ks, kz) in enumerate(kts):
                                nc.tensor.matmul(
                                    psF[:, :fo], lhsT=_pk(cur[ki][:], m),
                                    rhs=Wt[(l, ki)][:],
                                    start=(ki == 0), stop=(ki == len(kts) - 1))
                            Fsb = gw.tile([PKW, 512], F16, tag="Fsb",
                                          name="Fsb", bufs=5)
                            nc.scalar.copy(Fsb[:, :fo], psF[:, :fo])
                            Fsbs.append(Fsb)
                        for mi, (ms, mz) in enumerate(_kt(fo)):
                            psZ = pp.tile([128, NPACK * PKW], F32, tag="Z",
                                          name="Z")
                            for m in range(NPACK):
                                nc.tensor.matmul(
                                    psZ[:mz, PKW * m:PKW * m + PKW],
                                    lhsT=Fsbs[m][:, ms:ms + mz],
                                    rhs=Gt[m][:], start=True, stop=skip_bv)
                                if not skip_bv:
                                    nc.tensor.matmul(
                                        psZ[:mz, PKW * m:PKW * m + PKW],
                                        lhsT=bv[l][:, ms:ms + mz],
                                        rhs=Gte[m][:], start=False, stop=True)
                            zt = gw.tile([128, NPACK * PKW], F16, tag="ztmp",
                                         name="ztmp")
                            nc.vector.tensor_copy(zt[:mz, :], psZ[:mz, :])
                            nc.vector.scalar_tensor_tensor(
                                nxt[mi][:mz, :], zt[:mz, :], 0.01,
                                zt[:mz, :], ALU.mult, ALU.max)
                        cur = nxt

                # -- bn5 + LayerNorm over features (transposed) --
                with tc.tile_pool(name=f"ps_ln{k}", bufs=2, space="PSUM") as pp, \
                     tc.tile_pool(name=f"lnw{k}", bufs=1) as lw:
                    for i in range(4):
                        nc.vector.tensor_scalar(
                            hT[i][:], hT[i][:], g5c[:, i:i + 1],
                            b5c[:, i:i + 1], ALU.mult, ALU.add)
                    psmu = pp.tile([1, BC * P], F32, tag="mu", name="psmu")
                    for i in range(4):
                        nc.tensor.matmul(psmu[:], lhsT=ones_col[:],
                                         rhs=hT[i][:], start=(i == 0),
                                         stop=(i == 3))
                    mu_r = lw.tile([1, BC * P], F32, tag="mu_r")
                    nc.scalar.mul(mu_r[:], psmu[:], 1.0 / 512.0)
                    pssq = pp.tile([1, BC * P], F32, tag="sq", name="pssq")
                    for i in range(4):
                        hsq = lw.tile([128, BC * P], F16, tag="hsq",
                                      name="hsq", bufs=2)
                        nc.vector.tensor_mul(hsq[:], hT[i][:], hT[i][:])
                        nc.tensor.matmul(pssq[:], lhsT=ones_col[:], rhs=hsq[:],
                                         start=(i == 0), stop=(i == 3))
                    var_r = lw.tile([1, BC * P], F32, tag="var_r")
                    nc.scalar.mul(var_r[:], pssq[:], 1.0 / 512.0)
                    t_r = lw.tile([1, BC * P], F32, tag="t_r")
                    nc.vector.tensor_mul(t_r[:], mu_r[:], mu_r[:])
                    nc.vector.tensor_sub(var_r[:], var_r[:], t_r[:])
                    nc.vector.tensor_scalar_add(var_r[:], var_r[:], LN_EPS)
                    rs_r = _newton_rsqrt(nc, lw, var_r[:], [1, BC * P], "lnr")
                    bco = {}
                    for nm, row in (("mu", mu_r), ("rs", rs_r)):
                        row16 = lw.tile([1, BC * P], F16, tag="r16" + nm)
                        nc.vector.tensor_copy(row16[:], row[:])
                        psb = pp.tile([128, BC * P], F32, tag="bc" + nm,
                                      name="psb")
                        nc.tensor.matmul(psb[:], lhsT=ones_row[:],
                                         rhs=row16[:], start=True, stop=True)
                        sb = lw.tile([128, BC * P], F16, tag="bcs" + nm,
                                     name="bcs")
                        nc.vector.tensor_copy(sb[:], psb[:])
                        bco[nm] = sb
                    for i in range(4):
                        nc.vector.tensor_sub(hT[i][:], hT[i][:], bco["mu"][:])
                        nc.vector.tensor_mul(hT[i][:], hT[i][:], bco["rs"][:])
                        nc.vector.tensor_scalar(
                            hT[i][:], hT[i][:], lngc[:, i:i + 1],
                            lnbc[:, i:i + 1], ALU.mult, ALU.add)

                # -- dense hyp-net (Wd1 streamed fp16 from HBM) --
                scal, nal = [], []
                with tc.tile_pool(name=f"ps_d{k}", bufs=1, space="PSUM") as pz, \
                     tc.tile_pool(name=f"ps_dt{k}", bufs=2, space="PSUM") as pt, \
                     tc.tile_pool(name=f"dw{k}", bufs=1) as dw:
                    psz1 = pz.tile([BC, 512], F32, tag="z1", name="psz1")
                    for t in range(P):
                        rg = ring.tile([128, 4 * 512], F16, tag="wd1",
                                       name="wd1")
                        src = d_Wd1[512 * t:512 * t + 512, :].rearrange(
                            "(c r) n -> r c n", r=128)
                        nc.sync.dma_start(
                            out=rg[:].rearrange("r (c n) -> r c n", c=4),
                            in_=src)
                        for c in range(4):
                            nc.tensor.matmul(
                                psz1[:], lhsT=_nd(hT[c][:], t),
                                rhs=rg[:, 512 * c:512 * c + 512],
                                start=(t == 0 and c == 0),
                                stop=(t == P - 1 and c == 3 and skip_bd))
                    if not skip_bd:
                        nc.tensor.matmul(psz1[:], lhsT=ones_row32[:, :BC],
                                         rhs=bdr[0][:], start=False, stop=True)

                    def dense_ln_lrelu(ps_ap, fdim, li):
                        zr = dw.tile([BC, 512], F32, tag="d_zr", name="zr")
                        nc.vector.tensor_copy(zr[:, :fdim], ps_ap)
                        mu = dw.tile([BC, 1], F32, tag="d_mu", name="mu")
                        nc.vector.tensor_reduce(mu[:], zr[:, :fdim],
                                                mybir.AxisListType.X, ALU.add)
                        nc.vector.tensor_scalar_mul(mu[:], mu[:], 1.0 / fdim)
                        sq = dw.tile([BC, 512], F32, tag="d_sq", name="sq")
                        nc.vector.tensor_mul(sq[:, :fdim], zr[:, :fdim],
                                             zr[:, :fdim])
                        vr = dw.tile([BC, 1], F32, tag="d_vr", name="vr")
                        nc.vector.tensor_reduce(vr[:], sq[:, :fdim],
                                                mybir.AxisListType.X, ALU.add)
                        nc.vector.tensor_scalar_mul(vr[:], vr[:], 1.0 / fdim)
                        mm = dw.tile([BC, 1], F32, tag="d_mm", name="mm")
                        nc.vector.tensor_mul(mm[:], mu[:], mu[:])
                        nc.vector.tensor_sub(vr[:], vr[:], mm[:])
                        nc.vector.tensor_scalar_add(vr[:], vr[:], LN_EPS)
                        rs = _newton_rsqrt(nc, dw, vr[:], [BC, 1], "dn")
                        z = dw.tile([BC, 512], F32, tag="d_z", name="z")
                        nc.vector.tensor_scalar(z[:, :fdim], zr[:, :fdim],
                                                mu[:], rs[:],
                                                ALU.subtract, ALU.mult)
                        if not skip_ld:
                            nc.vector.tensor_mul(z[:, :fdim], z[:, :fdim],
                                                 ldg[li][:, :fdim])
                            nc.vector.tensor_add(z[:, :fdim], z[:, :fdim],
                                                 ldb[li][:, :fdim])
                        nc.vector.scalar_tensor_tensor(
                            z[:, :fdim], z[:, :fdim], 0.01, z[:, :fdim],
                            ALU.mult, ALU.max)
                        return z

                    def transpose_cols(z_sb, fdim):
                        outs = []
                        for ci, (cs, cz) in enumerate(_kt(fdim)):
                            pty = pt.tile([128, BC], F32, tag="zt", name="pty")
                            nc.tensor.transpose(pty[:cz, :],
                                                z_sb[:, cs:cs + cz],
                                                ident32[:BC, :BC])
                            zz = dw.tile([128, BC], F32, tag="d_zt", bufs=4,
                                         name=f"zz{ci}")
                            nc.vector.tensor_copy(zz[:cz, :], pty[:cz, :])
                            outs.append((zz, cz))
                        return outs

                    z1 = dense_ln_lrelu(psz1[:], 512, 0)
                    z1t = transpose_cols(z1, 512)
                    psz2 = pz.tile([BC, 256], F32, tag="z2", name="psz2")
                    for ci, (zz, cz) in enumerate(z1t):
                        nc.tensor.matmul(psz2[:], lhsT=zz[:cz, :],
                                         rhs=Wd2t[ci][:], start=(ci == 0),
                                         stop=(ci == 3 and skip_bd))
                    if not skip_bd:
                        nc.tensor.matmul(psz2[:], lhsT=ones_row32[:, :BC],
                                         rhs=bdr[1][:], start=False, stop=True)
                    z2 = dense_ln_lrelu(psz2[:], 256, 1)
                    z2t = transpose_cols(z2, 256)
                    psz3 = pz.tile([BC, 128], F32, tag="z3", name="psz3")
                    for ci, (zz, cz) in enumerate(z2t):
                        nc.tensor.matmul(psz3[:], lhsT=zz[:cz, :],
                                         rhs=Wd3t[ci][:], start=(ci == 0),
                                         stop=(ci == 1 and skip_bd))
                    if not skip_bd:
                        nc.tensor.matmul(psz3[:], lhsT=ones_row32[:, :BC],
                                         rhs=bdr[2][:], start=False, stop=True)
                    z3 = dense_ln_lrelu(psz3[:], 128, 2)
                    z3t = transpose_cols(z3, 128)
                    psfc = pz.tile([BC, 4], F32, tag="fc", name="psfc")
                    nc.tensor.matmul(psfc[:], lhsT=z3t[0][0][:128, :],
                                     rhs=Wfct[:], start=True, stop=False)
                    nc.tensor.matmul(psfc[:], lhsT=ones_row32[:, :BC],
                                     rhs=bdr[3][:], start=False, stop=True)
                    hyp = dw.tile([BC, 4], F32, tag="hyp", name="hyp")
                    nc.scalar.activation(hyp[:], psfc[:], AF.Sigmoid)
                    nc.vector.tensor_scalar(hyp[:], hyp[:], 0.9999, 1e-4,
                                            ALU.min, ALU.max)
                    for m in range(NPACK):
                        pss = pt.tile([PKW, 4], F32, tag="scal", name="pss")
                        nc.tensor.matmul(pss[:],
                                         lhsT=sel[:, PKW * m:PKW * m + PKW],
                                         rhs=hyp[:], start=True, stop=True)
                        sc = sw.tile([PKW, 4], F32, tag="sc", bufs=4,
                                     name=f"sc{m}")
                        nc.vector.tensor_copy(sc[:], pss[:])
                        na = sw.tile([PKW, 1], F32, tag="na", bufs=4,
                                     name=f"na{m}")
                        nc.vector.tensor_scalar_mul(na[:], sc[:, 0:1], -1.0)
                        scal.append(sc)
                        nal.append(na)

                # -- ADMM update (natural layout) --
                with tc.tile_pool(name=f"ps_ad{k}", bufs=2, space="PSUM") as pp, \
                     tc.tile_pool(name=f"adw{k}", bufs=2) as aw:
                    for m in range(NPACK):
                        g = g_t[m]
                        sg = aw.tile([PKW, N], F32, tag="sg", name="sg")
                        nc.scalar.activation(sg[:], y_nat[m][:], AF.Sign)
                        nc.vector.scalar_tensor_tensor(
                            g[:], sg[:], scal[m][:, 1:2], g[:],
                            ALU.mult, ALU.add)
                        nc.vector.scalar_tensor_tensor(
                            g[:], U_nat[m][:], snb[:, m:m + 1], g[:],
                            ALU.mult, ALU.add)
                        nc.vector.scalar_tensor_tensor(
                            g[:], dl_nat[m][:], scal[m][:, 2:3], g[:],
                            ALU.mult, ALU.add)
                        nc.vector.tensor_scalar(g[:], g[:], 10.0, -10.0,
                                                ALU.min, ALU.max)
                        nc.vector.scalar_tensor_tensor(
                            y_nat[m][:], g[:], nal[m][:], y_nat[m][:],
                            ALU.mult, ALU.add)
                        nc.vector.tensor_scalar(y_nat[m][:], y_nat[m][:],
                                                100.0, -100.0,
                                                ALU.min, ALU.max)
                        nc.sync.dma_start(
                            out=d_out[k, PKW * m:PKW * m + PKW, :],
                            in_=y_nat[m][:])
                        psd = pp.tile([PKW, N], F32, tag="dl", name="psd")
                        nc.tensor.matmul(psd[:], lhsT=Lt[m][:],
                                         rhs=y_nat[m][:], start=True,
                                         stop=True)
                        nc.vector.tensor_scalar(dl_nat[m][:], psd[:],
                                                20.0, -20.0, ALU.min, ALU.max)
                        nc.vector.scalar_tensor_tensor(
                            U_nat[m][:], dl_nat[m][:], scal[m][:, 3:4],
                            U_nat[m][:], ALU.mult, ALU.add)
                        nc.vector.tensor_scalar(U_nat[m][:], U_nat[m][:],
                                                100.0, -100.0,
                                                ALU.min, ALU.max)
                        if k < K - 1:
                            for j, (js, jz) in enumerate(KT200):
                                pty = pp.tile([128, PKW], F32, tag="ytr",
                                              name="pty2")
                                nc.tensor.transpose(
                                    pty[:jz, :], y_nat[m][:, js:js + jz],
                                    ident32[:PKW, :PKW])
                                nc.vector.tensor_copy(
                                    _pk(yT[j][:], m), pty[:jz, :])
                sw.release()
    nc.compile()
    return nc


_NC_CACHE = {}
LAST_RESULTS = None


def _host_inputs(inputs):
    """Build all per-core DRAM arrays (numpy). Returns (in_maps, flags)."""
    f32 = np.float32
    f16 = np.float16
    BN_SCALE = f32(1.0) / np.sqrt(f32(1.0) + f32(1e-5))
    b_in = np.ascontiguousarray(np.asarray(inputs['b'], f32)[..., 0])
    A0 = np.ascontiguousarray(np.asarray(inputs['A'], f32)[0])
    edge = np.asarray(inputs['edge_index'])
    y0 = np.ascontiguousarray(np.asarray(inputs['y0'], f32)[..., 0])
    U0 = np.ascontiguousarray(np.asarray(inputs['U0'], f32)[..., 0])
    d0 = np.ascontiguousarray(np.asarray(inputs['delta0'], f32)[..., 0])

    Ws = [np.asarray(inputs['W%d' % i], f32) for i in range(1, 6)]
    bs = [np.asarray(inputs['b%d' % i], f32) for i in range(1, 6)]
    gs = [np.asarray(inputs['g%d' % i], f32) * BN_SCALE for i in range(1, 6)]
    bes = [np.asarray(inputs['be%d' % i], f32) for i in range(1, 6)]
    Wf = [Ws[0]] + [gs[l - 1][:, None] * Ws[l] for l in range(1, 5)]
    vs = [np.zeros(FOUT[0], f32)] + [(bes[l - 1] @ Ws[l]).astype(f32)
                                     for l in range(1, 5)]
    lds = [(np.asarray(inputs['ld%dg' % i], f32),
            np.asarray(inputs['ld%db' % i], f32)) for i in (1, 2, 3)]
    bds = [np.asarray(inputs['bd%d' % i], f32) for i in (1, 2, 3)]

    flags = dict(
        skip_bv=bool(all(np.all(bs[l] == 0) and np.all(vs[l] == 0)
                         for l in range(5))),
        skip_ld=bool(all(np.all(g == 1) and np.all(bb == 0)
                         for g, bb in lds)),
        skip_bd=bool(all(np.all(bd == 0) for bd in bds)),
    )

    AtA = np.einsum('pmi,pmj->pij', A0, A0).astype(f32)       # [P,N,N]
    Atb_full = np.einsum('pmn,bpm->bpn', A0, b_in).astype(f32)  # [B,P,N]
    shared = {'AtA32': np.ascontiguousarray(
                  AtA.transpose(1, 0, 2).reshape(N, P * N)),
              'ident': np.eye(128, dtype=f16),
              'Wd1': np.asarray(inputs['Wd1'], f32).astype(f16),
              'Wd2': np.asarray(inputs['Wd2'], f32),
              'Wd3': np.asarray(inputs['Wd3'], f32),
              'Wfc': np.asarray(inputs['Wfc'], f32),
              'bd1r': bds[0][None, :],
              'bd2r': bds[1][None, :],
              'bd3r': bds[2][None, :],
              'bfcr': np.asarray(inputs['bfc'], f32)[None, :],
              'gs5c': np.ascontiguousarray(gs[4].reshape(4, 128).T),
              'be5c': np.ascontiguousarray(bes[4].reshape(4, 128).T),
              'lngc': np.ascontiguousarray(
                  np.asarray(inputs['ln_g'], f32).reshape(4, 128).T),
              'lnbc': np.ascontiguousarray(
                  np.asarray(inputs['ln_b'], f32).reshape(4, 128).T)}
    for l in range(5):
        shared[f'W{l+1}f'] = np.ascontiguousarray(Wf[l]).astype(f16)
        shared[f'bv{l+1}'] = np.stack([bs[l], vs[l]]).astype(f16)
    for i, nm in ((0, 'ld1'), (1, 'ld2'), (2, 'ld3')):
        g, bb = lds[i]
        shared[nm + 'gb'] = np.broadcast_to(g, (BC, g.size)).copy()
        shared[nm + 'bb'] = np.broadcast_to(bb, (BC, bb.size)).copy()
    selm = np.zeros((BC, NPACK * PKW), f32)
    for m in range(NPACK):
        for bl in range(2):
            selm[2 * m + bl, PKW * m + 50 * bl:PKW * m + 50 * bl + 50] = 1.0
    shared['sel'] = selm

    in_maps = []
    for c in range(NC_CORES):
        sl = slice(BC * c, BC * c + BC)
        d = dict(shared)
        Atb_c = Atb_full[sl]                                   # [BC,P,N]
        d['Atbn'] = Atb_c.reshape(NPACK * PKW, N).copy()
        d['AtbT'] = np.ascontiguousarray(
            Atb_c.transpose(2, 0, 1).reshape(N, BC * P)).astype(f16)
        d['y0T'] = np.ascontiguousarray(
            y0[sl].transpose(2, 0, 1).reshape(N, P * BC))
        d['y0n'] = y0[sl].reshape(NPACK * PKW, N).copy()
        d['U0n'] = U0[sl].reshape(NPACK * PKW, N).copy()
        d['d0n'] = d0[sl].reshape(NPACK * PKW, N).copy()
        GtT = np.zeros((NPACK, 102, PKW), f32)
        LtT = np.zeros((NPACK, PKW, PKW), f32)
        snbm = np.zeros((PKW, NPACK), f32)
        for m in range(NPACK):
            for bl in range(2):
                bg = BC * c + 2 * m + bl
                s, dd = edge[bg, 0], edge[bg, 1]
                cnt = np.zeros((P, P), np.int64)
                np.add.at(cnt, (dd, s), 1)
                deg = (cnt.sum(1) + 1).astype(f32)
                nb = cnt.sum(0).astype(f32)
                G = (cnt.astype(f32)
                     / np.sqrt(deg[:, None] * deg[None, :]).astype(f32))
                G[np.arange(P), np.arange(P)] += (f32(1.0) / deg)
                L = 2.0 * (np.diag(nb) - cnt.astype(f32))
                r0 = 50 * bl
                GtT[m, r0:r0 + 50, r0:r0 + 50] = G.T
                GtT[m, 101, r0:r0 + 50] = G.sum(1)
                LtT[m, r0:r0 + 50, r0:r0 + 50] = L.T.astype(f32)
                snbm[r0:r0 + 50, m] = nb
        GtT[:, 100, :] = 1.0
        d['GhatT'] = GtT[:, :PKW, :].reshape(NPACK * PKW, PKW).astype(f16)
        d['GhatE'] = GtT[:, PKW:102, :].reshape(NPACK * 2, PKW).astype(f16)
        d['LdT'] = LtT.reshape(NPACK * PKW, PKW).copy()
        d['snb'] = snbm
        in_maps.append(d)
    return in_maps, flags


def kernel(**inputs):
    in_maps, flags = _host_inputs(inputs)
    key = tuple(sorted(flags.items()))
    if key not in _NC_CACHE:
        _NC_CACHE[key] = build_nc(**flags)
    nc = _NC_CACHE[key]
    res = bass_utils.run_bass_kernel_spmd(nc, in_maps,
                                          core_ids=list(range(NC_CORES)))
    global LAST_RESULTS
    LAST_RESULTS = res
    out = np.empty((K, B, P, N, 1), np.float32)
    for c in range(NC_CORES):
        ys = res.results[c]['Ys'].reshape(K, BC, P, N)
        out[:, BC * c:BC * c + BC] = ys[..., None]
    return out
